# revision 20
# baseline (speedup 1.0000x reference)
"""Multi-head causal attention on 8 TRN2 NeuronCores.

Problem: B=4, T=2048, D=1024, H=16 heads of 64. Sharding: core c handles
batch c//2 and head-group c%2 (8 heads = 512 cols of the concat dim). Each
core computes its partial output projection o_g @ Wo_g^T; the host sums the
two partials per batch and adds the bias.

Mixed precision: the Q/K path runs in fp8 e4m3 (x and Wq/Wk cast on the
host, weights scaled by 64 into the fp8 normal range; quantization noise in
the scores washes out through the softmax average). The V path and output
projection stay bf16 (early causal rows have no averaging to hide noise).
Q/K projections use fp8 DoubleRow matmuls (k-tile pairs, 256-deep
contraction per instruction); score matmuls are plain fp8.

Per-core attention (fp32 accumulation):
  per head pair: scoresT[s, q] produced as a per-head 512-col chunk stream
  so every exp activation on ScalarE reads a full [128,1024] PSUM tile
  (uniform, minimal instruction count); exp writes PT[s, q] bf16; causal =
  multiply the diagonal 128-block by an upper-triangular 0/1 mask after exp
  (on Pool). PV runs LAG stripes behind scores so the PE stream never
  parks on the scores->exp->mask->PV chain: out[q, 0:65|65:130] = P @ V_aug
  accumulated over s-tiles; col 64 of each half is the softmax denominator.
  Normalize with a per-partition reciprocal, DMA-transpose [128,128] into
  oT[hd, t]. proj: partial[t, :] = oT^T @ WoT_g bf16, drained on ScalarE,
  fused into the last pair's PV stream.
"""

import numpy as np
import ml_dtypes
from contextlib import ExitStack

import concourse.mybir as mybir
import concourse.tile as tile
from concourse import bacc
from concourse.bass_utils import run_bass_kernel_spmd

F32 = mybir.dt.float32
BF16 = mybir.dt.bfloat16
FP8 = mybir.dt.float8e4
DR = mybir.MatmulPerfMode.DoubleRow

B, T, D, H = 4, 2048, 1024, 16
HD = 64          # head dim
HG = 8           # heads per core
GW = HG * HD     # 512, group width
NT = T // 128    # 16 t-tiles
NK = D // 128    # 8 d-tiles
N_CORES = 8
WS = 64.0        # host weight scale into fp8 normal range

# ragged PT stripe offsets: stripe j holds cols q=128j..2048
_PT_OFF = [2048 * j - 64 * j * (j - 1) for j in range(NT + 1)]
PT_LEN = _PT_OFF[NT]  # 17408


def _build():
    nc = bacc.Bacc("TRN2", target_bir_lowering=False, debug=False,
                   num_devices=N_CORES)
    x8_d = nc.dram_tensor("x8", [D, T], FP8, kind="ExternalInput").ap()
    xb_d = nc.dram_tensor("xb", [D, T], BF16, kind="ExternalInput").ap()
    wq_d = nc.dram_tensor("wq", [D, GW], FP8, kind="ExternalInput").ap()
    wk_d = nc.dram_tensor("wk", [D, GW], FP8, kind="ExternalInput").ap()
    wv_d = nc.dram_tensor("wv", [D, GW], BF16, kind="ExternalInput").ap()
    wo_d = nc.dram_tensor("woT", [GW, D], BF16, kind="ExternalInput").ap()
    tri_d = nc.dram_tensor("tri", [128, 128], BF16, kind="ExternalInput").ap()
    out_d = nc.dram_tensor("out", [T, D], F32, kind="ExternalOutput").ap()

    with tile.TileContext(nc) as tc, ExitStack() as ctx:
        perm = ctx.enter_context(tc.tile_pool(name="perm", bufs=1))
        psA = ctx.enter_context(tc.tile_pool(name="psA", bufs=2, space="PSUM"))
        psB = ctx.enter_context(tc.tile_pool(name="psB", bufs=2, space="PSUM"))
        ps_o = ctx.enter_context(tc.tile_pool(name="ps_o", bufs=2, space="PSUM"))

        tri = perm.tile([128, 128], BF16, tag="tri")
        nc.sync.dma_start(tri[:], tri_d[:])

        # Q/K fp8: planes 0-3 = q of pair m, 4-7 = k of pair m.
        # Head 2m+hh sits at partitions 64*hh.
        qkz = perm.tile([128, 8, T], FP8, tag="qkz")
        vsb = perm.tile([128, NT, HG * (HD + 1)], BF16, tag="vsb")
        wob = perm.tile([128, 4, D], BF16, tag="wob")
        oT = perm.tile([128, 4, T], BF16, tag="oT")

        # ones columns for V_aug
        vcols = vsb.rearrange("p j (h c) -> p j h c", c=HD + 1)
        nc.vector.memset(vcols[:, :, :, HD:HD + 1], 1.0)

        # per-head score-chunk pieces: cut at every 512 of the stream (psA
        # tile grid) and at every stripe boundary
        cuts = sorted(set(range(0, PT_LEN + 1, 512)) | set(_PT_OFF))
        pieces = []  # (a, b, j) with stripe j
        for a, bnd in zip(cuts[:-1], cuts[1:]):
            j = next(jj for jj in range(NT)
                     if _PT_OFF[jj] <= a < _PT_OFF[jj + 1])
            pieces.append((a, bnd, j))

        def scores_stripe(pair, pts, st, j):
            """score matmul pieces + full-tile exps for both heads of one
            stripe. st = per-head psA stream-tile state."""
            m = pair
            for hh in range(2):
                base = 64 * hh
                for (a, bnd, jj) in pieces:
                    if jj != j:
                        continue
                    if a % 1024 == 0:
                        st[hh] = psA.tile([128, 1024], F32, tag="psA",
                                          name=f"s{pair}_{hh}_{a}")
                    qa = 128 * j + (a - _PT_OFF[j])
                    nc.tensor.matmul(
                        st[hh][:, a % 1024:a % 1024 + (bnd - a)],
                        qkz[base:base + 64, 4 + m,
                            128 * j:128 * (j + 1)],
                        qkz[base:base + 64, m, qa:qa + (bnd - a)],
                        start=True, stop=True)
                    if bnd % 1024 == 0:
                        nc.scalar.activation(
                            pts[hh][:, bnd - 1024:bnd],
                            st[hh][:],
                            mybir.ActivationFunctionType.Exp,
                            scale=0.125 / (WS * WS))

        def mask_stripe(pair, pts, j):
            o0 = _PT_OFF[j]
            for hh in range(2):
                nc.gpsimd.tensor_mul(pts[hh][:, o0:o0 + 128],
                                     pts[hh][:, o0:o0 + 128], tri[:])

        def pv_i(pair, pts, i, smp, after_i=None):
            """PV + normalize + DMA-transpose into oT for one q-tile"""
            if True:
                po = ps_o.tile([128, 2 * (HD + 1)], F32, tag="po")
                for hh in range(2):
                    h = 2 * pair + hh
                    c0 = (HD + 1) * hh
                    pt = pts[hh]
                    for j in range(i + 1):
                        nc.tensor.matmul(
                            po[:, c0:c0 + HD + 1],
                            pt[:, _PT_OFF[j] + 128 * (i - j):
                               _PT_OFF[j] + 128 * (i - j) + 128],
                            vsb[:, j, (HD + 1) * h:(HD + 1) * (h + 1)],
                            start=(j == 0), stop=(j == i))
                recip = smp.tile([128, 2], F32, tag="recip")
                pov = po.rearrange("p (h c) -> p h c", c=HD + 1)
                nc.vector.reciprocal(recip[:], pov[:, :, HD])
                onat = smp.tile([128, 128], BF16, tag="onat")
                for hh in range(2):
                    c0 = (HD + 1) * hh
                    nc.vector.tensor_scalar_mul(
                        onat[:, 64 * hh:64 * hh + 64],
                        po[:, c0:c0 + HD], recip[:, hh:hh + 1])
                nc.sync.dma_start(oT[:, pair, 128 * i:128 * (i + 1)],
                                  onat[:], transpose=True)
                if after_i is not None:
                    after_i(i)

        with tc.tile_pool(name="ph1", bufs=1) as ph1, \
             tc.tile_pool(name="ptp", bufs=2) as ptp, \
             tc.tile_pool(name="sm", bufs=8) as smp, \
             tc.tile_pool(name="outp", bufs=2) as outp:
            x8 = ph1.tile([128, NK, T], FP8, tag="x8")
            xb = ph1.tile([128, NK, T], BF16, tag="xb")
            wqb = ph1.tile([128, NK, GW], FP8, tag="wqb")
            wkb = ph1.tile([128, NK, GW], FP8, tag="wkb")
            wvb = ph1.tile([128, NK, GW], BF16, tag="wvb")

            qs = [nc.sync, nc.scalar, nc.gpsimd]
            # priority order: x8 + wq/wk (unblock Q/K prep), then the first
            # t-chunk of xb + wv (unblock early v tiles), then the rest
            for k in range(NK):
                qs[k % 3].dma_start(x8[:, k, 0:512],
                                    x8_d[128 * k:128 * (k + 1), 0:512])
                qs[(k + 1) % 3].dma_start(wqb[:, k, :],
                                          wq_d[128 * k:128 * (k + 1), :])
                qs[(k + 2) % 3].dma_start(wkb[:, k, :],
                                          wk_d[128 * k:128 * (k + 1), :])
            for c in range(1, 4):
                for k in range(NK):
                    qs[(k + c) % 3].dma_start(
                        x8[:, k, 512 * c:512 * (c + 1)],
                        x8_d[128 * k:128 * (k + 1), 512 * c:512 * (c + 1)])
            for k in range(NK):
                qs[k % 3].dma_start(xb[:, k, 0:512],
                                    xb_d[128 * k:128 * (k + 1), 0:512])
                qs[(k + 1) % 3].dma_start(wvb[:, k, :],
                                          wv_d[128 * k:128 * (k + 1), :])
            for c in range(1, 4):
                for k in range(NK):
                    qs[(k + c) % 3].dma_start(
                        xb[:, k, 512 * c:512 * (c + 1)],
                        xb_d[128 * k:128 * (k + 1), 512 * c:512 * (c + 1)])
            for k in range(4):
                qs[k % 3].dma_start(wob[:, k, :], wo_d[128 * k:128 * (k + 1), :])

            # ---- Q/K projections (fp8 DoubleRow over k-pairs) ----
            def qkT_mtile(m, cs=range(4)):
                for c in cs:  # t chunks of 512
                    for (wbt, dst) in ((wqb, m), (wkb, 4 + m)):
                        ps = psB.tile([128, 512], F32, tag="psB")
                        for kp in range(4):
                            nc.tensor.matmul(
                                ps[:],
                                wbt[:, 2 * kp:2 * kp + 2,
                                    128 * m:128 * (m + 1)],
                                x8[:, 2 * kp:2 * kp + 2,
                                   512 * c:512 * (c + 1)],
                                start=(kp == 0), stop=(kp == 3),
                                perf_mode=DR)
                        nc.vector.tensor_copy(
                            qkz[:, dst, 512 * c:512 * (c + 1)], ps[:])

            def v_jtile(j):
                ps = psB.tile([128, 512], F32, tag="psB")
                for k in range(NK):
                    nc.tensor.matmul(ps[:],
                                     xb[:, k, 128 * j:128 * (j + 1)],
                                     wvb[:, k, :],
                                     start=(k == 0), stop=(k == NK - 1))
                nc.vector.tensor_copy(vcols[:, j, :, :HD], ps[:])

            qkT_mtile(0)

            def proj_i(i):
                ost = outp.tile([128, D], F32, tag="ost", name=f"ost{i}")
                for n in range(2):
                    ps = psB.tile([128, 512], F32, tag="psB")
                    for k in range(4):
                        nc.tensor.matmul(ps[:],
                                         oT[:, k, 128 * i:128 * (i + 1)],
                                         wob[:, k, 512 * n:512 * (n + 1)],
                                         start=(k == 0), stop=(k == 3))
                    nc.vector.tensor_copy(ost[:, 512 * n:512 * (n + 1)],
                                          ps[:])
                qs[i % 3].dma_start(out_d[128 * i:128 * (i + 1), 0:512],
                                    ost[:, 0:512])
                qs[(i + 1) % 3].dma_start(out_d[128 * i:128 * (i + 1), 512:D],
                                          ost[:, 512:D])

            qkT_mtile(1)

            # ---- attention head pairs ----
            LAG = 2   # stripes of slack between scores and their PV
            PLAG = 3  # extra tiles between a PV's transpose and its proj
            for pair in range(4):
                pts = [ptp.tile([128, PT_LEN], BF16, tag="pt",
                                name=f"pt{pair}_{hh}") for hh in range(2)]
                st = {}
                for j in range(NT):
                    scores_stripe(pair, pts, st, j)
                    if j >= 1:
                        mask_stripe(pair, pts, j - 1)
                    if j >= LAG:
                        pv_i(pair, pts, j - LAG, smp)
                        if pair == 3 and j >= LAG + PLAG:
                            proj_i(j - LAG - PLAG)
                    # filler after PV so its DVE normalize is not queued
                    # behind these drains (po buffer recycling)
                    if pair == 0:
                        v_jtile(j)
                    elif pair < 3 and j % 4 == 3:
                        qkT_mtile(pair + 1, cs=[j // 4])
                mask_stripe(pair, pts, NT - 1)
                for j in range(NT - LAG, NT):
                    pv_i(pair, pts, j, smp)
                    if pair == 3:
                        proj_i(j - PLAG)
                if pair == 3:
                    for i in range(NT - PLAG, NT):
                        proj_i(i)

    nc.compile()
    return nc


_NC_CACHE = None


def _get_nc():
    global _NC_CACHE
    if _NC_CACHE is None:
        _NC_CACHE = _build()
    return _NC_CACHE


def _prep_in_maps(x, Wq, Wk, Wv, Wo):
    bf = ml_dtypes.bfloat16
    f8 = ml_dtypes.float8_e4m3
    tri = np.triu(np.ones((128, 128), dtype=bf))
    in_maps = []
    for c in range(N_CORES):
        b, g = c // 2, c % 2
        hsl = slice(HG * g, HG * (g + 1))
        xT = np.ascontiguousarray(x[b].T)
        in_maps.append({
            "x8": xT.astype(f8),
            "xb": xT.astype(bf),
            "wq": np.ascontiguousarray(
                (Wq[hsl] * WS).transpose(1, 0, 2).reshape(D, GW)).astype(f8),
            "wk": np.ascontiguousarray(
                (Wk[hsl] * WS).transpose(1, 0, 2).reshape(D, GW)).astype(f8),
            "wv": np.ascontiguousarray(
                Wv[hsl].transpose(1, 0, 2).reshape(D, GW)).astype(bf),
            "woT": np.ascontiguousarray(
                Wo[:, GW * g:GW * (g + 1)].T).astype(bf),
            "tri": tri,
        })
    return in_maps


def kernel(x, Wq, Wk, Wv, Wo, bo, _trace=False, _tmpdir=None):
    nc = _get_nc()
    x = np.asarray(x, dtype=np.float32)
    bo = np.asarray(bo, dtype=np.float32)
    in_maps = _prep_in_maps(x, np.asarray(Wq, np.float32),
                            np.asarray(Wk, np.float32),
                            np.asarray(Wv, np.float32),
                            np.asarray(Wo, np.float32))
    res = run_bass_kernel_spmd(nc, in_maps, core_ids=list(range(N_CORES)),
                               trace=_trace, tmpdir=_tmpdir)
    out = np.empty((B, T, D), dtype=np.float32)
    for b in range(B):
        out[b] = res.results[2 * b]["out"].astype(np.float32) \
            + res.results[2 * b + 1]["out"].astype(np.float32) + bo
    if _trace:
        return out, res
    return out


# revision 21
# speedup vs baseline: 1.0195x; 1.0195x over previous
"""Multi-head causal attention on 8 TRN2 NeuronCores.

Problem: B=4, T=2048, D=1024, H=16 heads of 64. Sharding: core c handles
batch c//2 and head-group c%2 (8 heads = 512 cols of the concat dim). Each
core computes its partial output projection o_g @ Wo_g^T; the host sums the
two partials per batch and adds the bias.

Mixed precision: the Q/K path runs in fp8 e4m3 (x and Wq/Wk cast on the
host, weights scaled by 64 into the fp8 normal range; quantization noise in
the scores washes out through the softmax average). The V path and output
projection stay bf16 (early causal rows have no averaging to hide noise).
Q/K projections use fp8 DoubleRow matmuls (k-tile pairs, 256-deep
contraction per instruction); score matmuls are plain fp8.

Per-core attention (fp32 accumulation):
  per head pair: scoresT[s, q] produced as a per-head 512-col chunk stream
  so every exp activation on ScalarE reads a full [128,1024] PSUM tile
  (uniform, minimal instruction count); exp writes PT[s, q] bf16; causal =
  multiply the diagonal 128-block by an upper-triangular 0/1 mask after exp
  (on Pool). PV runs LAG stripes behind scores so the PE stream never
  parks on the scores->exp->mask->PV chain: out[q, 0:65|65:130] = P @ V_aug
  accumulated over s-tiles; col 64 of each half is the softmax denominator.
  Normalize with a per-partition reciprocal, DMA-transpose [128,128] into
  oT[hd, t]. proj: partial[t, :] = oT^T @ WoT_g bf16, drained on ScalarE,
  fused into the last pair's PV stream.
"""

import numpy as np
import ml_dtypes
from contextlib import ExitStack

import concourse.mybir as mybir
import concourse.tile as tile
from concourse import bacc
from concourse.bass_utils import run_bass_kernel_spmd

F32 = mybir.dt.float32
BF16 = mybir.dt.bfloat16
FP8 = mybir.dt.float8e4
DR = mybir.MatmulPerfMode.DoubleRow

B, T, D, H = 4, 2048, 1024, 16
HD = 64          # head dim
HG = 8           # heads per core
GW = HG * HD     # 512, group width
NT = T // 128    # 16 t-tiles
NK = D // 128    # 8 d-tiles
N_CORES = 8
WS = 64.0        # host weight scale into fp8 normal range

# ragged PT stripe offsets: stripe j holds cols q=128j..2048
_PT_OFF = [2048 * j - 64 * j * (j - 1) for j in range(NT + 1)]
PT_LEN = _PT_OFF[NT]  # 17408


def _build():
    nc = bacc.Bacc("TRN2", target_bir_lowering=False, debug=False,
                   num_devices=N_CORES)
    x8_d = nc.dram_tensor("x8", [D, T], FP8, kind="ExternalInput").ap()
    xb_d = nc.dram_tensor("xb", [D, T], BF16, kind="ExternalInput").ap()
    wq_d = nc.dram_tensor("wq", [D, GW], FP8, kind="ExternalInput").ap()
    wk_d = nc.dram_tensor("wk", [D, GW], FP8, kind="ExternalInput").ap()
    wv_d = nc.dram_tensor("wv", [D, GW], BF16, kind="ExternalInput").ap()
    wo_d = nc.dram_tensor("woT", [GW, D], BF16, kind="ExternalInput").ap()
    tri_d = nc.dram_tensor("tri", [128, 128], BF16, kind="ExternalInput").ap()
    out_d = nc.dram_tensor("out", [T, D], F32, kind="ExternalOutput").ap()

    with tile.TileContext(nc) as tc, ExitStack() as ctx:
        perm = ctx.enter_context(tc.tile_pool(name="perm", bufs=1))
        psA = ctx.enter_context(tc.tile_pool(name="psA", bufs=2, space="PSUM"))
        psB = ctx.enter_context(tc.tile_pool(name="psB", bufs=2, space="PSUM"))
        ps_o = ctx.enter_context(tc.tile_pool(name="ps_o", bufs=2, space="PSUM"))

        tri = perm.tile([128, 128], BF16, tag="tri")
        nc.sync.dma_start(tri[:], tri_d[:])

        # Q/K fp8: planes 0-3 = q of pair m, 4-7 = k of pair m.
        # Head 2m+hh sits at partitions 64*hh.
        qkz = perm.tile([128, 8, T], FP8, tag="qkz")
        vsb = perm.tile([128, NT, HG * (HD + 1)], BF16, tag="vsb")
        wob = perm.tile([128, 4, D], BF16, tag="wob")
        oT = perm.tile([128, 4, T], BF16, tag="oT")

        # ones columns for V_aug
        vcols = vsb.rearrange("p j (h c) -> p j h c", c=HD + 1)
        nc.vector.memset(vcols[:, :, :, HD:HD + 1], 1.0)

        # per-head score-chunk pieces: cut at every 512 of the stream (psA
        # tile grid) and at every stripe boundary
        cuts = sorted(set(range(0, PT_LEN + 1, 512)) | set(_PT_OFF))
        pieces = []  # (a, b, j) with stripe j
        for a, bnd in zip(cuts[:-1], cuts[1:]):
            j = next(jj for jj in range(NT)
                     if _PT_OFF[jj] <= a < _PT_OFF[jj + 1])
            pieces.append((a, bnd, j))

        def scores_stripe(pair, pts, st, j):
            """score matmul pieces + full-tile exps for both heads of one
            stripe. st = per-head psA stream-tile state."""
            m = pair
            for hh in range(2):
                base = 64 * hh
                for (a, bnd, jj) in pieces:
                    if jj != j:
                        continue
                    if a % 1024 == 0:
                        st[hh] = psA.tile([128, 1024], F32, tag="psA",
                                          name=f"s{pair}_{hh}_{a}")
                    qa = 128 * j + (a - _PT_OFF[j])
                    nc.tensor.matmul(
                        st[hh][:, a % 1024:a % 1024 + (bnd - a)],
                        qkz[base:base + 64, 4 + m,
                            128 * j:128 * (j + 1)],
                        qkz[base:base + 64, m, qa:qa + (bnd - a)],
                        start=True, stop=True)
                    if bnd % 1024 == 0:
                        nc.scalar.activation(
                            pts[hh][:, bnd - 1024:bnd],
                            st[hh][:],
                            mybir.ActivationFunctionType.Exp,
                            scale=0.125 / (WS * WS))

        def mask_stripe(pair, pts, j):
            o0 = _PT_OFF[j]
            for hh in range(2):
                nc.gpsimd.tensor_mul(pts[hh][:, o0:o0 + 128],
                                     pts[hh][:, o0:o0 + 128], tri[:])

        def pv_i(pair, pts, i, smp, after_i=None):
            """PV + normalize + DMA-transpose into oT for one q-tile"""
            if True:
                po = ps_o.tile([128, 2 * (HD + 1)], F32, tag="po")
                for hh in range(2):
                    h = 2 * pair + hh
                    c0 = (HD + 1) * hh
                    pt = pts[hh]
                    for j in range(i + 1):
                        nc.tensor.matmul(
                            po[:, c0:c0 + HD + 1],
                            pt[:, _PT_OFF[j] + 128 * (i - j):
                               _PT_OFF[j] + 128 * (i - j) + 128],
                            vsb[:, j, (HD + 1) * h:(HD + 1) * (h + 1)],
                            start=(j == 0), stop=(j == i))
                recip = smp.tile([128, 2], F32, tag="recip")
                pov = po.rearrange("p (h c) -> p h c", c=HD + 1)
                nc.vector.reciprocal(recip[:], pov[:, :, HD])
                onat = smp.tile([128, 128], BF16, tag="onat")
                for hh in range(2):
                    c0 = (HD + 1) * hh
                    nc.vector.tensor_scalar_mul(
                        onat[:, 64 * hh:64 * hh + 64],
                        po[:, c0:c0 + HD], recip[:, hh:hh + 1])
                nc.sync.dma_start(oT[:, pair, 128 * i:128 * (i + 1)],
                                  onat[:], transpose=True)
                if after_i is not None:
                    after_i(i)

        with tc.tile_pool(name="ph1", bufs=1) as ph1, \
             tc.tile_pool(name="ptp", bufs=2) as ptp, \
             tc.tile_pool(name="sm", bufs=8) as smp, \
             tc.tile_pool(name="outp", bufs=2) as outp:
            x8 = ph1.tile([128, NK, T], FP8, tag="x8")
            xb = ph1.tile([128, NK, T], BF16, tag="xb")
            wqb = ph1.tile([128, NK, GW], FP8, tag="wqb")
            wkb = ph1.tile([128, NK, GW], FP8, tag="wkb")
            wvb = ph1.tile([128, NK, GW], BF16, tag="wvb")

            qs = [nc.sync, nc.scalar, nc.gpsimd]
            # priority order: x8 + wq/wk (unblock Q/K prep), then the first
            # t-chunk of xb + wv (unblock early v tiles), then the rest
            for k in range(NK):
                qs[k % 3].dma_start(x8[:, k, 0:512],
                                    x8_d[128 * k:128 * (k + 1), 0:512])
                qs[(k + 1) % 3].dma_start(wqb[:, k, :],
                                          wq_d[128 * k:128 * (k + 1), :])
                qs[(k + 2) % 3].dma_start(wkb[:, k, :],
                                          wk_d[128 * k:128 * (k + 1), :])
            for c in range(1, 4):
                for k in range(NK):
                    qs[(k + c) % 3].dma_start(
                        x8[:, k, 512 * c:512 * (c + 1)],
                        x8_d[128 * k:128 * (k + 1), 512 * c:512 * (c + 1)])
            for k in range(NK):
                qs[k % 3].dma_start(xb[:, k, 0:512],
                                    xb_d[128 * k:128 * (k + 1), 0:512])
                qs[(k + 1) % 3].dma_start(wvb[:, k, :],
                                          wv_d[128 * k:128 * (k + 1), :])
            for c in range(1, 4):
                for k in range(NK):
                    qs[(k + c) % 3].dma_start(
                        xb[:, k, 512 * c:512 * (c + 1)],
                        xb_d[128 * k:128 * (k + 1), 512 * c:512 * (c + 1)])
            for k in range(4):
                qs[k % 3].dma_start(wob[:, k, :], wo_d[128 * k:128 * (k + 1), :])

            # ---- Q/K projections (fp8 DoubleRow over k-pairs) ----
            def qkT_mtile(m, cs=range(4)):
                for c in cs:  # t chunks of 512
                    for (wbt, dst) in ((wqb, m), (wkb, 4 + m)):
                        ps = psB.tile([128, 512], F32, tag="psB")
                        for kp in range(4):
                            nc.tensor.matmul(
                                ps[:],
                                wbt[:, 2 * kp:2 * kp + 2,
                                    128 * m:128 * (m + 1)],
                                x8[:, 2 * kp:2 * kp + 2,
                                   512 * c:512 * (c + 1)],
                                start=(kp == 0), stop=(kp == 3),
                                perf_mode=DR)
                        nc.vector.tensor_copy(
                            qkz[:, dst, 512 * c:512 * (c + 1)], ps[:])

            def v_jtile(j):
                ps = psB.tile([128, 512], F32, tag="psB")
                for k in range(NK):
                    nc.tensor.matmul(ps[:],
                                     xb[:, k, 128 * j:128 * (j + 1)],
                                     wvb[:, k, :],
                                     start=(k == 0), stop=(k == NK - 1))
                nc.vector.tensor_copy(vcols[:, j, :, :HD], ps[:])

            qkT_mtile(0)

            def proj_i(i):
                ost = outp.tile([128, D], F32, tag="ost", name=f"ost{i}")
                for n in range(2):
                    ps = psB.tile([128, 512], F32, tag="psB")
                    for k in range(4):
                        nc.tensor.matmul(ps[:],
                                         oT[:, k, 128 * i:128 * (i + 1)],
                                         wob[:, k, 512 * n:512 * (n + 1)],
                                         start=(k == 0), stop=(k == 3))
                    nc.vector.tensor_copy(ost[:, 512 * n:512 * (n + 1)],
                                          ps[:])
                qs[i % 3].dma_start(out_d[128 * i:128 * (i + 1), :], ost[:])

            qkT_mtile(1)

            # ---- attention head pairs ----
            LAG = 2   # stripes of slack between scores and their PV
            PLAG = 2  # extra tiles between a PV's transpose and its proj
            for pair in range(4):
                pts = [ptp.tile([128, PT_LEN], BF16, tag="pt",
                                name=f"pt{pair}_{hh}") for hh in range(2)]
                st = {}
                for j in range(NT):
                    scores_stripe(pair, pts, st, j)
                    if j >= 1:
                        mask_stripe(pair, pts, j - 1)
                    if j >= LAG:
                        pv_i(pair, pts, j - LAG, smp)
                        if pair == 3 and j >= LAG + PLAG:
                            proj_i(j - LAG - PLAG)
                    # filler after PV so its DVE normalize is not queued
                    # behind these drains (po buffer recycling)
                    if pair == 0:
                        v_jtile(j)
                    elif pair < 3 and j % 4 == 3:
                        qkT_mtile(pair + 1, cs=[j // 4])
                mask_stripe(pair, pts, NT - 1)
                for j in range(NT - LAG, NT):
                    pv_i(pair, pts, j, smp)
                    if pair == 3:
                        proj_i(j - PLAG)
                if pair == 3:
                    for i in range(NT - PLAG, NT):
                        proj_i(i)

    nc.compile()
    return nc


_NC_CACHE = None


def _get_nc():
    global _NC_CACHE
    if _NC_CACHE is None:
        _NC_CACHE = _build()
    return _NC_CACHE


def _prep_in_maps(x, Wq, Wk, Wv, Wo):
    bf = ml_dtypes.bfloat16
    f8 = ml_dtypes.float8_e4m3
    tri = np.triu(np.ones((128, 128), dtype=bf))
    in_maps = []
    for c in range(N_CORES):
        b, g = c // 2, c % 2
        hsl = slice(HG * g, HG * (g + 1))
        xT = np.ascontiguousarray(x[b].T)
        in_maps.append({
            "x8": xT.astype(f8),
            "xb": xT.astype(bf),
            "wq": np.ascontiguousarray(
                (Wq[hsl] * WS).transpose(1, 0, 2).reshape(D, GW)).astype(f8),
            "wk": np.ascontiguousarray(
                (Wk[hsl] * WS).transpose(1, 0, 2).reshape(D, GW)).astype(f8),
            "wv": np.ascontiguousarray(
                Wv[hsl].transpose(1, 0, 2).reshape(D, GW)).astype(bf),
            "woT": np.ascontiguousarray(
                Wo[:, GW * g:GW * (g + 1)].T).astype(bf),
            "tri": tri,
        })
    return in_maps


def kernel(x, Wq, Wk, Wv, Wo, bo, _trace=False, _tmpdir=None):
    nc = _get_nc()
    x = np.asarray(x, dtype=np.float32)
    bo = np.asarray(bo, dtype=np.float32)
    in_maps = _prep_in_maps(x, np.asarray(Wq, np.float32),
                            np.asarray(Wk, np.float32),
                            np.asarray(Wv, np.float32),
                            np.asarray(Wo, np.float32))
    res = run_bass_kernel_spmd(nc, in_maps, core_ids=list(range(N_CORES)),
                               trace=_trace, tmpdir=_tmpdir)
    out = np.empty((B, T, D), dtype=np.float32)
    for b in range(B):
        out[b] = res.results[2 * b]["out"].astype(np.float32) \
            + res.results[2 * b + 1]["out"].astype(np.float32) + bo
    if _trace:
        return out, res
    return out
